# revision 47
# baseline (speedup 1.0000x reference)
"""Trainium2 Bass kernel for the Haar-mask MLP (histogram_binning).

Every Haar interval edge is a multiple of 2^-10, so the reference's masks --
and therefore the entire MLP output -- depend only on u = floor(t * 1024)
(1024 values).  The network collapses to a 1024x3 lookup table computed once
on host from the tiny weights; the device evaluates it at 16384 points/core.

GpSimd indirect gathers are SBUF-read-latency bound (~28 ns/index, ~57 us
per core), so the LUT is instead evaluated with matmuls over transposed
step masks.  With u = 16*h + l (h in [0,64), l in [0,16)) and a DOUBLE
TELESCOPE (the stationary is first-differenced along both h and l on host):

  out[f, x] = sum_l [l_x >= l] * D'[(l,f), x]
  D'[(l,f), x] = sum_a [w_x >= a] * ddLUT[a, (l,f)]      (w = u/16, fp16-exact)

  - u computed exactly in the natural [128p, s] layout (round-then-adjust),
    in 3 column segments so the first quarter's pipeline starts early,
  - w (fp16) and l (uint8) rows flattened to DRAM and broadcast-DMA'd
    across partitions in per-quarter pieces on 3 DMA queues,
  - Mh[128, x] = is_ge(w_bcast, a_p) via tensor_scalar with a per-partition
    AP scalar; mm1 uses a block-diagonal [128, 96] stationary so one moving
    pass serves both 64-row chunks -> D' in PSUM,
  - fused DVE scalar_tensor_tensor: E = (l_bcast is_ge l_p) * D' straight
    from PSUM (no evacuation),
  - mm2: ONES[96, 6] reduces E -> out PSUM tiles packed 2 quarters per
    [38, 2048] tile (partition bases 0/32), ScalarE evacuates fp16 halves.

fp16 end-to-end (integer values and LUT deltas are fp16-safe): rel err vs
the fp32 reference ~1.3e-3, well under the 2e-2 gate.  HW exec ~44-47 us
vs the 74 us gather baseline; ~20 us of that is fixed NEFF pre/postamble
(engine bring-up + full semaphore-file reset), ~6 us HBM-bound broadcast
pipeline-fill, ~13 us DVE mask/select stream (PSUM-operand 1x floor).

The import-time subprocess patch upgrades walrus to the time-aware
post-scheduler (--policy=3 + dynamic-AP dep opt), worth ~1 us; the last
block runs at 512-col granularity so the final out-DMA (which gates the
fixed postlude) issues as early as possible.
"""

from contextlib import ExitStack

import numpy as np

import concourse.tile as tile
from concourse import bacc, mybir
from concourse.bass_utils import run_bass_kernel_spmd

import concourse.bass_utils as _bu

if not getattr(_bu, "_walrus_patch", False):
    _orig_check_call = _bu.subprocess.check_call

    def _patched_check_call(argv, *a, **kw):
        if isinstance(argv, list) and argv and "walrus" in str(argv[0]):
            argv = ["--policy=3" if x == "--policy=0" else x for x in argv]
            argv = argv + ["--enable-dynamic-AP-dep-opt"]
        return _orig_check_call(argv, *a, **kw)

    _bu.subprocess.check_call = _patched_check_call
    _bu._walrus_patch = True

N_CORES = 8
B, T, F = 16, 8192, 3
N = B * T                    # 131072 total elements
NPC = N // N_CORES           # 16384 per neuron core
P = 128
NH, NL = 64, 16              # u = 16*h + l
G = 2                        # chunks per core (64 h-rows each)
CC = NPC // G                # 8192 cols per chunk
NQ = 4                       # col quarters for pipelining
QC = CC // NQ                # 2048 cols per quarter
NB = 2                       # 1024-col blocks per quarter
BC = QC // NB                # 1024
MM = 512                     # moving cols per matmul

GATHER_IMPL = "mm"           # legacy knob (test.py may set it); unused
RUN_KWARGS = {}
LAST_RESULTS = None
_CACHE = {}


def _build_lut(W1, b1, W2, b2, W3, b3):
    """MLP output for each of the 1024 half-interval bins, fp32 math."""
    u = np.arange(1024)
    acc = np.zeros((1024, W1.shape[1]), np.float32)
    for j in range(10):
        k = u >> (10 - j)
        idx = (1 << j) - 1 + k
        sign = np.where((u >> (9 - j)) & 1 == 0, np.float32(1), np.float32(-1))
        acc = acc + sign[:, None] * W1[idx]
    h = np.maximum(acc + b1, np.float32(0))
    h = np.maximum(h @ W2 + b2, np.float32(0))
    return (h @ W3 + b3).astype(np.float32)     # (1024, 3)


def _build_nc():
    nc = bacc.Bacc("TRN2", target_bir_lowering=False, debug=False,
                   enable_asserts=False, num_devices=N_CORES)
    f32 = mybir.dt.float32
    f16 = mybir.dt.float16
    i32 = mybir.dt.int32
    OP = mybir.AluOpType

    t_d = nc.dram_tensor("t", [P, NPC // P], f32, kind="ExternalInput")
    cpk16_d = nc.dram_tensor("cpk16", [P, 102], f16, kind="ExternalInput")
    cpk32_d = nc.dram_tensor("cpk32", [P, 2], f32, kind="ExternalInput")
    out_d = nc.dram_tensor("out", [2, 38, QC], f16, kind="ExternalOutput")
    hrow_d = nc.dram_tensor("hrow", [G, CC], f16, kind="Internal")
    lrow_d = nc.dram_tensor("lrow", [G, CC], mybir.dt.uint8, kind="Internal")

    S = NPC // P             # 128 slots per partition in natural layout
    HC = CC // 2             # 4096 cols per half
    R96 = G * NL * F         # 96 rows for U_l / D / E

    with tile.TileContext(nc) as tc, ExitStack() as ctx:
        cpool = ctx.enter_context(tc.tile_pool(name="c", bufs=1))
        qpool = ctx.enter_context(tc.tile_pool(name="q", bufs=1))
        dpool = ctx.enter_context(tc.tile_pool(name="dps", bufs=2, space="PSUM"))
        opool = ctx.enter_context(tc.tile_pool(name="ops", bufs=1, space="PSUM"))
        spool = ctx.enter_context(tc.tile_pool(name="s", bufs=4))

        # ---- constants (2 packed DMAs on the gpsimd queue) ----
        cpk16 = cpool.tile([P, 102], f16, tag="cpk16")
        cpk32 = cpool.tile([P, 2], f32, tag="cpk32")
        nc.gpsimd.dma_start(cpk16[:], cpk16_d[:, :])
        nc.gpsimd.dma_start(cpk32[:], cpk32_d[:, :])
        bdlut = cpk16[:, 0:96]
        ones = cpk16[0:R96, 96:102]
        aconst = cpk32[:, 0:1]
        lconst = cpk32[0:R96, 1:2]

        t_sb = cpool.tile([P, S], f32, tag="t")
        nc.sync.dma_start(t_sb[:, 0:32], t_d[:, 0:32])
        nc.scalar.dma_start(t_sb[:, 32:64], t_d[:, 32:64])
        nc.scalar.dma_start(t_sb[:, 64:128], t_d[:, 64:128])

        # ---- u-compute per half + marshaling DMAs issued as data lands ----
        # w = u/16 = h + l/16 (fp16-exact, 10 bits); step masks use is_ge on w
        w16 = cpool.tile([P, S], f16, tag="w16")
        l8 = cpool.tile([P, S], mybir.dt.uint8, tag="l8")
        uh = cpool.tile([P, CC], f16, tag="uh")
        ul = cpool.tile([R96, CC], mybir.dt.uint8, tag="ul")
        # u-compute in 3 segments (q0 first for fastest pipeline head)
        segs = [(0, 32, [0]), (32, 64, [1]), (64, 128, [2, 3])]
        for si, (s0, s1, qlist) in enumerate(segs):
            sl = slice(s0, s1)
            W = s1 - s0
            csl = slice(64 * s0, 64 * s1)              # chunk cols of segment
            v1k = qpool.tile([P, W], f32, tag=f"v1k_{si}")
            iv2 = qpool.tile([P, W], i32, tag=f"iv2_{si}")
            fv2 = qpool.tile([P, W], f32, tag=f"fv2_{si}")
            adj2 = qpool.tile([P, W], f32, tag=f"adj2_{si}")
            uf = qpool.tile([P, W], f32, tag=f"uf_{si}")
            ui3 = qpool.tile([P, W], i32, tag=f"ui3_{si}")
            li3 = qpool.tile([P, W], i32, tag=f"li3_{si}")
            # u = exact floor(1024 t)
            nc.vector.tensor_scalar(v1k[:], t_sb[:, sl], 1024.0, None, OP.mult)
            nc.vector.tensor_copy(iv2[:], v1k[:])
            nc.vector.tensor_copy(fv2[:], iv2[:])
            nc.vector.tensor_tensor(adj2[:], fv2[:], v1k[:], OP.is_gt)
            nc.vector.tensor_tensor(uf[:], fv2[:], adj2[:], OP.subtract)
            nc.vector.tensor_scalar(w16[:, sl], uf[:], 1.0 / 16.0, None,
                                    OP.mult)
            nc.sync.dma_start(hrow_d.ap()[0:2, csl], w16[:, sl])
            for qq in qlist:
                qcs = slice(QC * qq, QC * (qq + 1))
                nc.sync.dma_start(uh[0:64, qcs],
                                  hrow_d.ap()[0:1, qcs].to_broadcast((64, QC)))
                nc.scalar.dma_start(uh[64:128, qcs],
                                    hrow_d.ap()[1:2, qcs].to_broadcast((64, QC)))
            nc.vector.tensor_copy(ui3[:], uf[:])
            nc.vector.tensor_scalar(li3[:], ui3[:], 15, None, OP.bitwise_and)
            nc.vector.tensor_copy(l8[:, sl], li3[:])
            nc.scalar.dma_start(lrow_d.ap()[0:2, csl], l8[:, sl])
            for qq in qlist:
                qcs = slice(QC * qq, QC * (qq + 1))
                nc.gpsimd.dma_start(ul[0:48, qcs],
                                    lrow_d.ap()[0:1, qcs].to_broadcast((48, QC)))
                nc.gpsimd.dma_start(ul[48:96, qcs],
                                    lrow_d.ap()[1:2, qcs].to_broadcast((48, QC)))

        # ---- interleaved steady state: DVE one quarter ahead on is_eq ----
        mh = cpool.tile([P, CC], f16, tag="mh")

        def emit_iseq(q):
            qsl = slice(QC * q, QC * (q + 1))
            nc.vector.tensor_scalar(mh[:, qsl], uh[:, qsl], aconst,
                                    None, OP.is_ge)

        def emit_mm1(q):
            dt = []
            for b in range(NB):
                dps = dpool.tile([R96, BC], mybir.dt.float32, tag="d")
                dt.append(dps)
                for m in range(BC // MM):
                    msl = slice(QC * q + BC * b + MM * m,
                                QC * q + BC * b + MM * (m + 1))
                    nc.tensor.matmul(dps[:, MM * m:MM * (m + 1)],
                                     bdlut, mh[:, msl],
                                     start=True, stop=True)
            return dt

        def emit_stt_one(q, b, dt):
            bsl = slice(QC * q + BC * b, QC * q + BC * (b + 1))
            e = spool.tile([R96, BC], f16, tag="e")
            nc.vector.scalar_tensor_tensor(
                e[:], ul[:, bsl], lconst, dt[:], OP.is_ge, OP.mult)
            return e

        def emit_stt(q, dtiles, q7_blocks=()):
            et = []
            for b in range(NB):
                bsl = slice(QC * q + BC * b, QC * q + BC * (b + 1))
                e = spool.tile([R96, BC], f16, tag="e")
                et.append(e)
                if b in q7_blocks:
                    ds = spool.tile([R96, BC], f16, tag="ds")
                    nc.scalar.copy(ds[:], dtiles[b][:])
                    msk = spool.tile([R96, BC], f16, tag="msk")
                    nc.vector.tensor_scalar(msk[:], ul[:, bsl], lconst,
                                            None, OP.is_ge)
                    nc.gpsimd.tensor_tensor(e[:], msk[:], ds[:], OP.mult)
                else:
                    nc.vector.scalar_tensor_tensor(
                        e[:], ul[:, bsl], lconst, dtiles[b][:],
                        OP.is_ge, OP.mult)
            return et

        def emit_mm2(q, etiles, oacc):
            ob = 32 * (q % 2)
            for b in range(NB):
                for m in range(BC // MM):
                    nc.tensor.matmul(
                        oacc[ob:ob + G * F,
                             BC * b + MM * m:BC * b + MM * (m + 1)],
                        ones, etiles[b][:, MM * m:MM * (m + 1)],
                        start=True, stop=True)

        def emit_pair_out(pair, oacc, half=None):
            if half is None:
                osb = spool.tile([38, QC], f16, tag="osb")
                nc.scalar.copy(osb[:], oacc[:])
                nc.sync.dma_start(out_d.ap()[pair, :, :], osb[:])
            else:
                hsl = slice(BC * half, BC * (half + 1))
                osb = spool.tile([38, BC], f16, tag=f"osbh{half}")
                nc.scalar.copy(osb[:], oacc[:, hsl])
                nc.sync.dma_start(out_d.ap()[pair, :, hsl], osb[:])

        emit_iseq(0)
        emit_iseq(1)
        oacc0 = opool.tile([38, QC], mybir.dt.float32, tag="oacc")
        d0 = emit_mm1(0)
        d1 = emit_mm1(1)
        # q0-b0 at 512-col granules: select starts after the first matmul
        e0f = []
        for m in range(BC // MM):
            ef = spool.tile([R96, MM], f16, tag=f"e0f_{m}")
            e0f.append(ef)
            nc.vector.scalar_tensor_tensor(
                ef[:], ul[:, MM * m:MM * (m + 1)], lconst,
                d0[0][:, MM * m:MM * (m + 1)], OP.is_ge, OP.mult)
        e0b1 = emit_stt_one(0, 1, d0[1])
        for m in range(BC // MM):
            nc.tensor.matmul(oacc0[0:G * F, MM * m:MM * (m + 1)],
                             ones, e0f[m][:], start=True, stop=True)
        for m in range(BC // MM):
            nc.tensor.matmul(oacc0[0:G * F, BC + MM * m:BC + MM * (m + 1)],
                             ones, e0b1[:, MM * m:MM * (m + 1)],
                             start=True, stop=True)
        emit_iseq(2)
        e1 = emit_stt(1, d1, q7_blocks=())
        ob0 = 32
        for m in range(BC // MM):
            nc.tensor.matmul(oacc0[ob0:ob0 + G * F, MM * m:MM * (m + 1)],
                             ones, e1[0][:, MM * m:MM * (m + 1)],
                             start=True, stop=True)
        emit_pair_out(0, oacc0, half=0)
        d2 = emit_mm1(2)
        for m in range(BC // MM):
            nc.tensor.matmul(
                oacc0[ob0:ob0 + G * F, BC + MM * m:BC + MM * (m + 1)],
                ones, e1[1][:, MM * m:MM * (m + 1)],
                start=True, stop=True)
        emit_pair_out(0, oacc0, half=1)
        emit_iseq(3)
        e2 = emit_stt(2, d2, q7_blocks=())
        oacc1 = opool.tile([38, QC], mybir.dt.float32, tag="oacc")
        d3 = emit_mm1(3)
        emit_mm2(2, e2, oacc1)
        # last quarter at fine granularity so the final out-DMA issues early
        ob = 32
        bsl0 = slice(QC * 3, QC * 3 + BC)
        e30 = spool.tile([R96, BC], f16, tag="e")
        nc.vector.scalar_tensor_tensor(
            e30[:], ul[:, bsl0], lconst, d3[0][:], OP.is_ge, OP.mult)
        for m in range(BC // MM):
            nc.tensor.matmul(oacc1[ob:ob + G * F, MM * m:MM * (m + 1)],
                             ones, e30[:, MM * m:MM * (m + 1)],
                             start=True, stop=True)
        emit_pair_out(1, oacc1, half=0)
        for m in range(BC // MM):
            msl = slice(MM * m, MM * (m + 1))
            e31 = spool.tile([R96, MM], f16, tag=f"e31_{m}")
            nc.vector.scalar_tensor_tensor(
                e31[:], ul[:, QC * 3 + BC + MM * m:QC * 3 + BC + MM * (m + 1)],
                lconst, d3[1][:, msl], OP.is_ge, OP.mult)
            nc.tensor.matmul(
                oacc1[ob:ob + G * F, BC + MM * m:BC + MM * (m + 1)],
                ones, e31[:], start=True, stop=True)
            osbq = spool.tile([38, MM], f16, tag=f"osbq_{m}")
            nc.scalar.copy(osbq[:], oacc1[:, BC + MM * m:BC + MM * (m + 1)])
            nc.sync.dma_start(
                out_d.ap()[1, :, BC + MM * m:BC + MM * (m + 1)], osbq[:])
    nc.compile()
    return nc


def _host_consts(lut):
    """Packed constants: cpk16 = [bdlut | ones], cpk32 = [aconst | lconst]."""
    lut3 = lut.reshape(NH, NL, F)
    d = lut3.copy()
    d[:, 1:, :] -= lut3[:, :-1, :]              # telescope along l
    d[1:, :, :] -= (d + np.cumsum(np.zeros_like(d), 0))[:-1, :, :] * 0  # noop
    dl = lut3.copy()
    dl[:, 1:, :] -= lut3[:, :-1, :]
    da = dl.copy()
    da[1:, :, :] -= dl[:-1, :, :]               # telescope along h (step masks)
    d2 = da.reshape(NH, NL * F)                 # col j = 3l + f
    cpk16 = np.zeros((P, 102), np.float16)
    for g in range(G):
        cpk16[64 * g:64 * g + 64, 48 * g:48 * g + 48] = d2
    for g in range(G):
        for l in range(NL):
            for f in range(F):
                cpk16[48 * g + 3 * l + f, 96 + 3 * g + f] = 1
    cpk32 = np.zeros((P, 2), np.float32)
    cpk32[:, 0] = np.arange(P) % 64
    cpk32[:G * NL * F, 1] = (np.arange(G * NL * F) % 48) // 3
    return cpk16, cpk32


def _host_t(t):
    """Core m natural tile: partition p slot s holds element
    8192*(p//64) + 2048*(s//32) + 32*(p%64) + (s%32) of the core's chunk."""
    tf = np.ascontiguousarray(np.asarray(t, np.float32)).reshape(N_CORES, NPC)
    # index array mapping (p, s) -> element; flatten segments: slots
    # [0,32) -> cols 32p'+s, [32,64) -> 2048+32p'+(s-32), [64,128) -> 4096+64p'+(s-64)
    p = np.arange(P)[:, None]
    s = np.arange(NPC // P)[None, :]
    e = np.where(
        s < 32, 8192 * (p // 64) + 32 * (p % 64) + s,
        np.where(s < 64, 8192 * (p // 64) + 2048 + 32 * (p % 64) + (s - 32),
                 8192 * (p // 64) + 4096 + 64 * (p % 64) + (s - 64)))
    return tf[:, e]                              # (N_CORES, 128, 128)


def _host_output(raw):
    """raw [2, 38, 2048] fp16: [pair, 32j + 3g+f, c] = elem 8192g + 2048(2p+j) + c."""
    r = raw.reshape(2, 38, QC)
    out = np.empty((G, NQ, QC, F), np.float32)
    for pair in range(2):
        for j in range(2):
            q = 2 * pair + j
            blk = r[pair, 32 * j:32 * j + 6, :]          # [6, 2048]
            out[:, q, :, :] = blk.reshape(G, F, QC).transpose(0, 2, 1)
    return out.reshape(NPC, F)


def kernel(t, W1, b1, W2, b2, W3, b3):
    global LAST_RESULTS
    if "nc" not in _CACHE:
        _CACHE["nc"] = _build_nc()
    nc = _CACHE["nc"]

    lut = _build_lut(np.asarray(W1, np.float32), np.asarray(b1, np.float32),
                     np.asarray(W2, np.float32), np.asarray(b2, np.float32),
                     np.asarray(W3, np.float32), np.asarray(b3, np.float32))
    cpk16, cpk32 = _host_consts(lut)
    tperm = _host_t(t)
    in_maps = [{"t": np.ascontiguousarray(tperm[m]),
                "cpk16": cpk16, "cpk32": cpk32}
               for m in range(N_CORES)]

    res = run_bass_kernel_spmd(nc, in_maps, list(range(N_CORES)), **RUN_KWARGS)
    LAST_RESULTS = res
    outs = [_host_output(res.results[m]["out"]) for m in range(N_CORES)]
    return np.concatenate(outs, axis=0).reshape(B, T, F).astype(np.float32)
